# revision 1
# baseline (speedup 1.0000x reference)
"""Distributed kNN (DkNN conformal credibility) on 8 TRN2 NeuronCores.

Math: the reference's per-layer normalize+center cancels for ranking ---
top-75 by EuclideanSquared of normalized-centered vectors == top-75 by
descending (q . t_n) / ||t_n||.  Each core ranks its 12500-row shard of the
training set, packs the train label into the low 4 mantissa bits of the
scaled score (ties in packed values then always share a label), selects a
local top-80 per query via max8/match_replace peeling, all-gathers the
8x80 candidates, and every core redundantly reduces to the global top-75,
class counts, conformal p-values, and argmax credibility.
"""

import os
import sys
import types

for _p in ("/opt/trn_rl_repo", "/root/.axon_site/_ro/trn_rl_repo"):
    if os.path.isdir(_p) and _p not in sys.path:
        sys.path.insert(0, _p)

import numpy as np

import concourse.bass as bass
import concourse.mybir as mybir
from concourse.tile import TileContext
from concourse.vector_clock import ScopedClock
from concourse.bass_utils import run_bass_kernel_spmd

# ---------------------------------------------------------------- constants
N_CORES = 8
L = 2
N_TRAIN = 100000
N_SHARD = N_TRAIN // N_CORES          # 12500
D = 512
B = 256
K = 75
C = 10
NB_CALI = 750
KD = D // 128                         # 4 contraction k-tiles
STRIP = 500                           # n-columns per strip (25 exact strips)
N_STRIPS = N_SHARD // STRIP           # 25
CHUNK_TOPK = 16                       # stage-1 candidates per strip (2x max8)
N_CAND = N_STRIPS * CHUNK_TOPK        # 400 stage-1 candidates per (layer,rg)
N_ROUNDS = 4                          # local peel rounds of 8 -> top-32
N_KEEP = N_ROUNDS * 8                 # 32 shipped per (layer,rg,row)
K_MERGE_ROUNDS = 10                   # global peel -> top-80 >= 75
NEG_INF = -3.0e38
RG = B // 128                         # 2 query row-groups

F32 = mybir.dt.float32
BF16 = mybir.dt.bfloat16
U32 = mybir.dt.uint32

MATMUL_DTYPE = F32                    # flips to float32r for the fast path


# ------------------------------------------------- tile tail-drain workaround
def _patched_drain_and_barrier(self, tick_clock, wait_clock):
    # walrus rejects >few sync waits on one instruction; the stock tail
    # drain aggregates every live semaphore.  Spread them over drains.
    nc = self.nc
    drain_inst = nc.sync.drain()
    wait_clock.add_sem_waits(
        drain_inst.ins, ScopedClock({None: tick_clock.global_clock})
    )
    si = drain_inst.ins.sync_info
    waits = list(si.on_wait) if si is not None else []
    if len(waits) > 1:
        si.on_wait = waits[:1]
        SyncInfo = type(si)
        for w in waits[1:]:
            wi = nc.sync.drain()
            wi.ins.sync_info = SyncInfo(on_wait=[w], on_update=[])

    nc.all_engine_barrier()
    assert self.sems is not None
    popped = nc._tile_sem_poison_stack.pop()
    assert popped is self._sem_poison
    nc.clear_and_free_semaphores(list(self.sems.allocated().values()))
    nc.all_engine_barrier()


TileContext._drain_and_barrier = _patched_drain_and_barrier

# Cap embedded sync-waits per instruction; hoist the excess onto injected
# same-engine NOPs placed immediately before (program order on one engine
# makes this equivalent).
_MAX_WAITS = 1
_orig_lower_ordered_insts = TileContext._lower_ordered_insts


def _redistribute_waits(self, ordered):
    nc = self.nc
    SyncInfo = None
    for bb_name, insts in ordered.items():
        new_list = []
        for inst in insts:
            si = getattr(inst, 'sync_info', None)
            waits = list(si.on_wait) if si is not None else []
            cap = 1 if not isinstance(inst, mybir.InstNoOp) else _MAX_WAITS
            if len(waits) > cap:
                if SyncInfo is None:
                    SyncInfo = type(si)
                keep = waits[:cap]
                extra = waits[cap:]
                si.on_wait = keep
                for j in range(0, len(extra), _MAX_WAITS):
                    nop = mybir.InstNoOp(
                        name=f"waitnop-{nc.next_id()}", ins=[], outs=[],
                        engine=inst.engine,
                    )
                    nop.sync_info = SyncInfo(
                        on_wait=extra[j:j + _MAX_WAITS], on_update=[]
                    )
                    nc.register_instruction(nop, overwrite=True)
                    new_list.append(nop)
            new_list.append(inst)
        insts[:] = new_list
    return _orig_lower_ordered_insts(self, ordered)


TileContext._lower_ordered_insts = _redistribute_waits


def _register_ntff_hook():
    """Optional: make run_bass_kernel_spmd(trace=True) work under axon."""
    if 'antenv.axon_hooks' not in sys.modules:
        m = types.ModuleType('antenv.axon_hooks')
        hook = [None]
        m.set_axon_ntff_profile_hook = lambda h: hook.__setitem__(0, h)
        m.get_axon_ntff_profile_hook = lambda: hook[0]
        sys.modules['antenv.axon_hooks'] = m
        try:
            import antenv
            antenv.axon_hooks = m
        except ImportError:
            pass
    try:
        from antenv.axon_hooks import set_axon_ntff_profile_hook
        from trn_agent_boot.trn_boot import _ntff_profile_via_ctypes
        h = _ntff_profile_via_ctypes('/opt/axon/libaxon_pjrt.so')
        if h is not None:
            set_axon_ntff_profile_hook(h)
    except Exception:
        pass


_register_ntff_hook()


# ------------------------------------------------------------- kernel build
def build_kernel():
    nc = bass.Bass("TRN2", num_devices=N_CORES)

    tHI = nc.declare_dram_parameter("tHI", [L, D, N_SHARD], BF16, isOutput=False)
    tLO = nc.declare_dram_parameter("tLO", [L, D, N_SHARD], BF16, isOutput=False)
    tN = nc.declare_dram_parameter("tN", [L, N_SHARD, D], F32, isOutput=False)
    ident = nc.declare_dram_parameter("ident", [128, 128], F32, isOutput=False)
    qHI = nc.declare_dram_parameter("qHI", [L, D, B], BF16, isOutput=False)
    qLO = nc.declare_dram_parameter("qLO", [L, D, B], BF16, isOutput=False)
    labels = nc.declare_dram_parameter("labels", [N_SHARD], U32, isOutput=False)
    cali = nc.declare_dram_parameter("cali", [NB_CALI], F32, isOutput=False)
    creds = nc.declare_dram_parameter("creds", [B, C], F32, isOutput=True)

    local_cand = nc.dram_tensor("local_cand", [L, RG, 128, N_KEEP], F32)
    gshared = nc.dram_tensor(
        "gshared", [N_CORES, L, RG, 128, N_KEEP], F32, addr_space="Shared"
    )

    with TileContext(nc) as tc, \
         tc.tile_pool(name="persist", bufs=1) as persist, \
         tc.tile_pool(name="tin", bufs=3) as tin_pool, \
         tc.tile_pool(name="tnat", bufs=4) as tnat_pool, \
         tc.tile_pool(name="nsq", bufs=2) as nsq_pool, \
         tc.tile_pool(name="ssc", bufs=3) as ssc_pool, \
         tc.tile_pool(name="pk", bufs=3) as pk_pool, \
         tc.tile_pool(name="pst", bufs=1, space="PSUM") as pst_pool, \
         tc.tile_pool(name="pss", bufs=7, space="PSUM") as pss_pool:

        # ---------------- setup
        labb = persist.tile([128, N_SHARD], U32)
        nc.sync.dma_start(out=labb[:], in_=labels[:].partition_broadcast(128))
        calib = persist.tile([128, NB_CALI], F32)
        nc.sync.dma_start(out=calib[:], in_=cali[:].partition_broadcast(128))
        idn = persist.tile([128, 128], F32)
        nc.sync.dma_start(out=idn[:], in_=ident[:])
        mask_hi = persist.tile([128, 1], U32)
        nc.vector.memset(mask_hi[:], 0xFFFFFFF0)
        mask_lo = persist.tile([128, 1], U32)
        nc.vector.memset(mask_lo[:], 0xF)

        qts = []
        for l in range(L):
            qh = persist.tile([128, KD, B], BF16, name=f"qth{l}", tag=f"qth{l}")
            nc.sync.dma_start(
                out=qh[:], in_=qHI[l].rearrange("(k p) b -> p k b", p=128)
            )
            ql = persist.tile([128, KD, B], BF16, name=f"qtl{l}", tag=f"qtl{l}")
            nc.sync.dma_start(
                out=ql[:], in_=qLO[l].rearrange("(k p) b -> p k b", p=128)
            )
            qts.append((qh, ql))

        cands = [
            [persist.tile([128, N_CAND], F32, name=f"cand{l}_{rg}", tag=f"cand{l}_{rg}") for rg in range(RG)]
            for l in range(L)
        ]
        wins = [
            [persist.tile([128, N_KEEP], F32, name=f"win{l}_{rg}", tag=f"win{l}_{rg}") for rg in range(RG)]
            for l in range(L)
        ]

        # ---------------- phase A: exact fp32 inverse norms, both layers
        # natural-layout pass, ACT Square+accum -> [n-part] norms, then
        # rsqrt, PE transpose, and flatten into a [1, N] row per layer.
        NT = (N_SHARD + 127) // 128          # 98 natural 128-row tiles
        nrm = persist.tile([128, L, NT], F32)
        nc.vector.memset(nrm[:], 1.0)
        invrows = []
        for l in range(L):
            for i in range(NT):
                r0 = i * 128
                rows = min(128, N_SHARD - r0)
                tn = tnat_pool.tile([128, D], F32, name="tn", tag="tn")
                nc.gpsimd.dma_start(out=tn[:rows, :], in_=tN[l, r0:r0 + rows, :])
                scr = nsq_pool.tile([128, D], F32, name="scr", tag="scr")
                if i % 2 == 0:
                    nc.scalar.activation(
                        scr[:rows, :], tn[:rows, :],
                        mybir.ActivationFunctionType.Square,
                        accum_out=nrm[:rows, l, i:i + 1],
                    )
                else:
                    nc.vector.scalar_tensor_tensor(
                        out=scr[:rows, :], in0=tn[:rows, :], scalar=1.0,
                        in1=tn[:rows, :],
                        op0=mybir.AluOpType.mult, op1=mybir.AluOpType.mult,
                        accum_out=nrm[:rows, l, i:i + 1],
                    )
            inb = persist.tile([128, NT], F32, name="inb", tag="inb")
            nc.vector.reciprocal(inb[:], nrm[:, l, :])
            nc.scalar.activation(
                inb[:], inb[:], mybir.ActivationFunctionType.Sqrt
            )
            pst = pst_pool.tile([NT, 128], F32, name="pst", tag="pst")
            nc.tensor.transpose(pst[:], inb[:], idn[:])
            invT = persist.tile([NT, 128], F32, name="invT", tag="invT")
            nc.scalar.copy(invT[:], pst[:])
            # full-width inverse-norm broadcast for this layer: one flatten
            # DMA into partition 0, then log2 partition doubling
            invfull = persist.tile(
                [128, NT * 128], F32, name="invfull", tag="invfull"
            )
            nc.scalar.dma_start(
                out=invfull[0:1, :].rearrange("o (p f) -> o p f", p=NT),
                in_=invT[:],
            )
            p = 1
            while p < 128:
                nc.scalar.dma_start(
                    out=invfull[p:2 * p, :], in_=invfull[0:p, :]
                )
                p *= 2
            invrows.append(invfull)

        # ---------------- main loop: per layer, per 500-column strip
        for l in range(L):
            for s in range(N_STRIPS):
                n0 = s * STRIP
                tin_h = tin_pool.tile([128, KD, STRIP], BF16, name="tin_h", tag="tin_h")
                nc.sync.dma_start(
                    out=tin_h[:],
                    in_=tHI[l, :, n0:n0 + STRIP].rearrange(
                        "(k p) n -> p k n", p=128
                    ),
                )
                tin_l = tin_pool.tile([128, KD, STRIP], BF16, name="tin_l", tag="tin_l")
                nc.sync.dma_start(
                    out=tin_l[:],
                    in_=tLO[l, :, n0:n0 + STRIP].rearrange(
                        "(k p) n -> p k n", p=128
                    ),
                )

                for rg in range(RG):
                    pss = pss_pool.tile([128, STRIP], F32)
                    qh, ql = qts[l]
                    bs = rg * 128
                    terms = []
                    for k in range(KD):
                        terms += [(qh[:, k, bs:bs + 128], tin_h[:, k, :]),
                                  (qh[:, k, bs:bs + 128], tin_l[:, k, :]),
                                  (ql[:, k, bs:bs + 128], tin_h[:, k, :]),
                                  (ql[:, k, bs:bs + 128], tin_l[:, k, :])]
                    for ti, (wa, xb) in enumerate(terms):
                        nc.tensor.matmul(
                            pss[:], wa, xb,
                            start=(ti == 0), stop=(ti == len(terms) - 1),
                        )
                    # scale-by-invnorm doubles as the PSUM eviction
                    pk = pk_pool.tile([128, STRIP], F32)
                    nc.vector.tensor_tensor(
                        out=pk[:], in0=pss[:],
                        in1=invrows[l][:, n0:n0 + STRIP],
                        op=mybir.AluOpType.mult,
                    )
                    nc.vector.scalar_tensor_tensor(
                        out=pk[:].bitcast(U32),
                        in0=pk[:].bitcast(U32),
                        scalar=mask_hi[:],
                        in1=labb[:, n0:n0 + STRIP],
                        op0=mybir.AluOpType.bitwise_and,
                        op1=mybir.AluOpType.bitwise_or,
                    )
                    # stage-1: top-8 of each 250-chunk (global-top-75
                    # members per 250-chunk ~ Poisson(0.19); P(>8) ~ 1e-12)
                    cd = cands[l][rg]
                    c0 = s * CHUNK_TOPK
                    half = STRIP // 2
                    nc.vector.max(out=cd[:, c0:c0 + 8], in_=pk[:, :half])
                    nc.vector.max(out=cd[:, c0 + 8:c0 + 16], in_=pk[:, half:])

            # local top-32 peel for this layer (overlaps next layer's strips)
            for rg in range(RG):
                cd, wn = cands[l][rg], wins[l][rg]
                for r in range(N_ROUNDS):
                    w8 = wn[:, r * 8:(r + 1) * 8]
                    nc.vector.max(out=w8, in_=cd[:])
                    if r < N_ROUNDS - 1:
                        nc.vector.match_replace(
                            out=cd[:], in_to_replace=w8,
                            in_values=cd[:], imm_value=NEG_INF,
                        )
                nc.sync.dma_start(out=local_cand[l, rg], in_=wn[:])

        # ---------------- all-gather candidates
        nc.gpsimd.collective_compute(
            "AllGather",
            mybir.AluOpType.bypass,
            replica_groups=[list(range(N_CORES))],
            ins=[local_cand[:]],
            outs=[gshared[:]],
        )

        # ---------------- replicated global merge
        for rg in range(RG):
            labw = persist.tile([128, L * K], U32, name=f"labw{rg}", tag=f"labw{rg}")
            for l in range(L):
                gcand = persist.tile([128, N_CORES * N_KEEP], F32, name=f"gcand{rg}_{l}", tag=f"gcand{rg}_{l}")
                nc.sync.dma_start(
                    out=gcand[:],
                    in_=gshared[:, l, rg, :, :].rearrange("c p k -> p c k"),
                )
                gwin = persist.tile([128, K_MERGE_ROUNDS * 8], F32, name=f"gwin{rg}_{l}", tag=f"gwin{rg}_{l}")
                for r in range(K_MERGE_ROUNDS):
                    w8 = gwin[:, r * 8:(r + 1) * 8]
                    nc.vector.max(out=w8, in_=gcand[:])
                    if r < K_MERGE_ROUNDS - 1:
                        nc.vector.match_replace(
                            out=gcand[:], in_to_replace=w8,
                            in_values=gcand[:], imm_value=NEG_INF,
                        )
                # labels of the global top-75
                nc.vector.tensor_scalar(
                    out=labw[:, l * K:(l + 1) * K],
                    in0=gwin[:, 0:K].bitcast(U32),
                    scalar1=mask_lo[:], scalar2=None,
                    op0=mybir.AluOpType.bitwise_and,
                )

            # labels as f32 values for is_equal comparisons
            labwf = persist.tile([128, L * K], F32, name=f"labwf{rg}", tag=f"labwf{rg}")
            nc.vector.tensor_copy(labwf[:], labw[:])
            # class counts -> v = 150 - count -> m = #{cali >= v}
            scr = persist.tile([128, L * K], F32, name=f"scr{rg}", tag=f"scr{rg}")
            vt = persist.tile([128, C], F32, name=f"vt{rg}", tag=f"vt{rg}")
            scr750 = persist.tile([128, NB_CALI], F32, name=f"scr750{rg}", tag=f"scr750{rg}")
            mge = persist.tile([128, C], F32, name=f"mge{rg}", tag=f"mge{rg}")
            mp = persist.tile([128, C], F32, name=f"mp{rg}", tag=f"mp{rg}")
            for c in range(C):
                cnt = vt[:, c:c + 1]
                nc.vector.tensor_scalar(
                    out=scr[:], in0=labwf[:], scalar1=float(c), scalar2=0.0,
                    op0=mybir.AluOpType.is_equal, op1=mybir.AluOpType.add,
                    accum_out=cnt,
                )
                # v = 150 - cnt  (in place)
                nc.vector.tensor_scalar(
                    out=cnt, in0=cnt, scalar1=-1.0, scalar2=float(L * K),
                    op0=mybir.AluOpType.mult, op1=mybir.AluOpType.add,
                )
                nc.vector.tensor_scalar(
                    out=scr750[:], in0=calib[:], scalar1=cnt,
                    scalar2=0.0, op0=mybir.AluOpType.is_ge,
                    op1=mybir.AluOpType.add,
                    accum_out=mge[:, c:c + 1],
                )
                # tie-break packing: mp = m*16 + (15 - c); argmax prefers
                # larger m then smaller class index, matching jnp.argmax
                nc.vector.tensor_scalar(
                    out=mp[:, c:c + 1], in0=mge[:, c:c + 1],
                    scalar1=16.0, scalar2=float(15 - c),
                    op0=mybir.AluOpType.mult, op1=mybir.AluOpType.add,
                )
            rmax = persist.tile([128, 1], F32, name=f"rmax{rg}", tag=f"rmax{rg}")
            nc.vector.tensor_reduce(
                out=rmax[:], in_=mp[:], axis=mybir.AxisListType.X,
                op=mybir.AluOpType.max,
            )
            mask = persist.tile([128, C], F32, name=f"mask{rg}", tag=f"mask{rg}")
            nc.vector.tensor_scalar(
                out=mask[:], in0=mp[:], scalar1=rmax[:],
                scalar2=None, op0=mybir.AluOpType.is_equal,
            )
            crd = persist.tile([128, C], F32, name=f"crd{rg}", tag=f"crd{rg}")
            nc.vector.tensor_tensor(
                out=crd[:], in0=mask[:], in1=mge[:],
                op=mybir.AluOpType.mult,
            )
            nc.vector.tensor_scalar(
                out=crd[:], in0=crd[:], scalar1=1.0 / NB_CALI, scalar2=None,
                op0=mybir.AluOpType.mult,
            )
            nc.sync.dma_start(out=creds[rg * 128:(rg + 1) * 128, :], in_=crd[:])

    return nc


_CACHE = {}


def _split_bf16(x):
    import ml_dtypes
    hi = x.astype(ml_dtypes.bfloat16)
    lo = (x - hi.astype(np.float32)).astype(ml_dtypes.bfloat16)
    return hi, lo


def _prep_inputs(train_feats, query_feats, train_labels, cali_nonconformity):
    train_feats = np.ascontiguousarray(train_feats, dtype=np.float32)
    query_feats = np.ascontiguousarray(query_feats, dtype=np.float32)
    labels = np.asarray(train_labels).astype(np.uint32)
    cali_f = np.asarray(cali_nonconformity).astype(np.float32)

    qT = np.ascontiguousarray(query_feats.transpose(0, 2, 1))  # [L, D, B]
    qhi, qlo = _split_bf16(qT)
    in_maps = []
    for i in range(N_CORES):
        sl = slice(i * N_SHARD, (i + 1) * N_SHARD)
        tn = np.ascontiguousarray(train_feats[:, sl, :])
        tT = np.ascontiguousarray(tn.transpose(0, 2, 1))
        thi, tlo = _split_bf16(tT)
        in_maps.append({
            "tHI": thi,
            "tLO": tlo,
            "tN": tn,
            "qHI": qhi,
            "qLO": qlo,
            "labels": np.ascontiguousarray(labels[sl]),
            "cali": cali_f,
            "ident": np.eye(128, dtype=np.float32),
        })
    return in_maps


def kernel(train_feats, query_feats, train_labels, cali_nonconformity,
           trace=False, **trace_kwargs):
    if "nc" not in _CACHE:
        _CACHE["nc"] = build_kernel()
    nc = _CACHE["nc"]
    in_maps = _prep_inputs(
        train_feats, query_feats, train_labels, cali_nonconformity
    )
    res = run_bass_kernel_spmd(
        nc, in_maps, list(range(N_CORES)), trace=trace, **trace_kwargs
    )
    _CACHE["last_result"] = res
    return np.asarray(res.results[0]["creds"], dtype=np.float32)



# revision 4
# speedup vs baseline: 1.7300x; 1.7300x over previous
"""Distributed kNN (DkNN conformal credibility) on 8 TRN2 NeuronCores.

Math: the reference's per-layer normalize+center cancels for ranking ---
top-75 by EuclideanSquared of normalized-centered vectors == top-75 by
descending q . (t_n/||t_n||).  The host pre-normalizes the train shard, so
the kernel is a pure score matmul + top-k funnel: each core scores its
12500-point shard against all 256 queries (fp32r matmul, exact fp32),
packs the train label into the low 4 mantissa bits of the score, selects
a local top-32 per query via max8/match_replace peeling, all-gathers the
8x32 candidates per layer (layer 0's gather overlaps layer 1's compute),
and every core redundantly reduces to the global top-75, class counts,
conformal p-values (via a host-precomputed 151-entry histogram of the
calibration array), and argmax credibility.
"""

import os
import sys
import types

for _p in ("/opt/trn_rl_repo", "/root/.axon_site/_ro/trn_rl_repo"):
    if os.path.isdir(_p) and _p not in sys.path:
        sys.path.insert(0, _p)

import numpy as np

import concourse.bass as bass
import concourse.mybir as mybir
from concourse.tile import TileContext
from concourse.vector_clock import ScopedClock
from concourse.bass_utils import run_bass_kernel_spmd

# ---------------------------------------------------------------- constants
N_CORES = 8
L = 2
N_TRAIN = 100000
N_SHARD = N_TRAIN // N_CORES          # 12500
D = 512
B = 256
K = 75
C = 10
NB_CALI = 750
KD = D // 128                         # 4 contraction k-tiles
STRIP = 500                           # n-columns per strip (25 exact strips)
N_STRIPS = N_SHARD // STRIP           # 25
CHUNK_TOPK = 16                       # stage-1 candidates per strip (2x max8)
N_CAND = N_STRIPS * CHUNK_TOPK        # 400 stage-1 candidates per (layer,rg)
N_ROUNDS = 4                          # local peel rounds of 8 -> top-32
N_KEEP = N_ROUNDS * 8                 # 32 shipped per (layer,rg,row)
K_MERGE_ROUNDS = 10                   # global peel -> top-80 >= 75
NEG_INF = -3.0e38
RG = B // 128                         # 2 query row-groups
NH = 151                              # histogram table size (L*K+1)

F32 = mybir.dt.float32
F32R = mybir.dt.float32r
BF16 = mybir.dt.bfloat16
U32 = mybir.dt.uint32

SCHEME = os.environ.get("KNN_SCHEME", "f32r")   # "f32r" | "bf16x3"


# ------------------------------------------------- tile tail-drain workaround
def _patched_drain_and_barrier(self, tick_clock, wait_clock):
    # walrus rejects >few sync waits on one instruction; the stock tail
    # drain aggregates every live semaphore.  Spread them over drains.
    nc = self.nc
    drain_inst = nc.sync.drain()
    wait_clock.add_sem_waits(
        drain_inst.ins, ScopedClock({None: tick_clock.global_clock})
    )
    si = drain_inst.ins.sync_info
    waits = list(si.on_wait) if si is not None else []
    if len(waits) > 1:
        si.on_wait = waits[:1]
        SyncInfo = type(si)
        for w in waits[1:]:
            wi = nc.sync.drain()
            wi.ins.sync_info = SyncInfo(on_wait=[w], on_update=[])

    nc.all_engine_barrier()
    assert self.sems is not None
    popped = nc._tile_sem_poison_stack.pop()
    assert popped is self._sem_poison
    nc.clear_and_free_semaphores(list(self.sems.allocated().values()))
    nc.all_engine_barrier()


TileContext._drain_and_barrier = _patched_drain_and_barrier

# Cap embedded sync-waits per instruction; hoist the excess onto injected
# same-engine NOPs placed immediately before (program order on one engine
# makes this equivalent).
_MAX_WAITS = 1
_orig_lower_ordered_insts = TileContext._lower_ordered_insts


def _redistribute_waits(self, ordered):
    nc = self.nc
    SyncInfo = None
    for bb_name, insts in ordered.items():
        new_list = []
        for inst in insts:
            si = getattr(inst, 'sync_info', None)
            waits = list(si.on_wait) if si is not None else []
            cap = 1 if not isinstance(inst, mybir.InstNoOp) else _MAX_WAITS
            if len(waits) > cap:
                if SyncInfo is None:
                    SyncInfo = type(si)
                keep = waits[:cap]
                extra = waits[cap:]
                si.on_wait = keep
                for j in range(0, len(extra), _MAX_WAITS):
                    nop = mybir.InstNoOp(
                        name=f"waitnop-{nc.next_id()}", ins=[], outs=[],
                        engine=inst.engine,
                    )
                    nop.sync_info = SyncInfo(
                        on_wait=extra[j:j + _MAX_WAITS], on_update=[]
                    )
                    nc.register_instruction(nop, overwrite=True)
                    new_list.append(nop)
            new_list.append(inst)
        insts[:] = new_list
    return _orig_lower_ordered_insts(self, ordered)


TileContext._lower_ordered_insts = _redistribute_waits


def _register_ntff_hook():
    """Optional: make run_bass_kernel_spmd(trace=True) work under axon."""
    if 'antenv.axon_hooks' not in sys.modules:
        m = types.ModuleType('antenv.axon_hooks')
        hook = [None]
        m.set_axon_ntff_profile_hook = lambda h: hook.__setitem__(0, h)
        m.get_axon_ntff_profile_hook = lambda: hook[0]
        sys.modules['antenv.axon_hooks'] = m
        try:
            import antenv
            antenv.axon_hooks = m
        except ImportError:
            pass
    try:
        from antenv.axon_hooks import set_axon_ntff_profile_hook
        from trn_agent_boot.trn_boot import _ntff_profile_via_ctypes
        h = _ntff_profile_via_ctypes('/opt/axon/libaxon_pjrt.so')
        if h is not None:
            set_axon_ntff_profile_hook(h)
    except Exception:
        pass


_register_ntff_hook()


# ------------------------------------------------------------- kernel build
def build_kernel(scheme):
    nc = bass.Bass("TRN2", num_devices=N_CORES)

    if scheme == "f32r":
        tS = nc.declare_dram_parameter(
            "tS", [L, N_STRIPS, 128, KD * STRIP], F32, isOutput=False)
        qW = nc.declare_dram_parameter("qW", [L, 128, KD * B], F32, isOutput=False)
    else:
        tHI = nc.declare_dram_parameter(
            "tHI", [L, N_STRIPS, 128, KD * STRIP], BF16, isOutput=False)
        tLO = nc.declare_dram_parameter(
            "tLO", [L, N_STRIPS, 128, KD * STRIP], BF16, isOutput=False)
        qHI = nc.declare_dram_parameter("qHI", [L, 128, KD * B], BF16, isOutput=False)
        qLO = nc.declare_dram_parameter("qLO", [L, 128, KD * B], BF16, isOutput=False)
    labels = nc.declare_dram_parameter("labels", [N_SHARD], U32, isOutput=False)
    htab = nc.declare_dram_parameter("htab", [NH], F32, isOutput=False)
    iotah = nc.declare_dram_parameter("iotah", [NH], F32, isOutput=False)
    creds = nc.declare_dram_parameter("creds", [B, C], F32, isOutput=True)

    local_cand = [
        nc.dram_tensor(f"local_cand{l}", [RG, 128, N_KEEP], F32) for l in range(L)
    ]
    gshared = [
        nc.dram_tensor(
            f"gshared{l}", [N_CORES, RG, 128, N_KEEP], F32, addr_space="Shared"
        )
        for l in range(L)
    ]

    with TileContext(nc) as tc, \
         tc.tile_pool(name="persist", bufs=1) as persist, \
         tc.tile_pool(name="tin", bufs=3) as tin_pool, \
         tc.tile_pool(name="pk", bufs=3) as pk_pool, \
         tc.tile_pool(name="pss", bufs=6, space="PSUM") as pss_pool:

        # ---------------- setup (all overlappable with early strips)
        labb = persist.tile([128, N_SHARD], U32)
        nc.gpsimd.dma_start(out=labb[:], in_=labels[:].partition_broadcast(128))
        hrow = persist.tile([128, NH], F32)
        nc.scalar.dma_start(out=hrow[:], in_=htab[:].partition_broadcast(128))
        irow = persist.tile([128, NH], F32)
        nc.scalar.dma_start(out=irow[:], in_=iotah[:].partition_broadcast(128))
        mask_hi = persist.tile([128, 1], U32)
        nc.vector.memset(mask_hi[:], 0xFFFFFFF0)
        mask_lo = persist.tile([128, 1], U32)
        nc.vector.memset(mask_lo[:], 0xF)

        qts = []
        for l in range(L):
            if scheme == "f32r":
                qt = persist.tile([128, KD, B], F32, name=f"qt{l}", tag=f"qt{l}")
                nc.sync.dma_start(out=qt[:], in_=qW[l])
                qts.append((qt,))
            else:
                qh = persist.tile([128, KD, B], BF16, name=f"qth{l}", tag=f"qth{l}")
                nc.sync.dma_start(out=qh[:], in_=qHI[l])
                ql = persist.tile([128, KD, B], BF16, name=f"qtl{l}", tag=f"qtl{l}")
                nc.sync.dma_start(out=ql[:], in_=qLO[l])
                qts.append((qh, ql))

        cands = [
            [persist.tile([128, N_CAND], F32, name=f"cand{l}_{rg}",
                          tag=f"cand{l}_{rg}") for rg in range(RG)]
            for l in range(L)
        ]
        wins = [
            [persist.tile([128, N_KEEP], F32, name=f"win{l}_{rg}",
                          tag=f"win{l}_{rg}") for rg in range(RG)]
            for l in range(L)
        ]

        # ---------------- main loop: per layer, per 500-column strip
        for l in range(L):
            for s in range(N_STRIPS):
                if scheme == "f32r":
                    tin = tin_pool.tile([128, KD * STRIP], F32, name="tin",
                                        tag="tin")
                    eng = nc.sync if s % 2 == 0 else nc.scalar
                    eng.dma_start(out=tin[:], in_=tS[l, s])
                else:
                    tin_h = tin_pool.tile([128, KD * STRIP], BF16, name="tin_h",
                                          tag="tin_h")
                    nc.sync.dma_start(out=tin_h[:], in_=tHI[l, s])
                    tin_l = tin_pool.tile([128, KD * STRIP], BF16, name="tin_l",
                                          tag="tin_l")
                    nc.scalar.dma_start(out=tin_l[:], in_=tLO[l, s])

                for rg in range(RG):
                    pss = pss_pool.tile([128, STRIP], F32)
                    bs = rg * 128
                    if scheme == "f32r":
                        (qt,) = qts[l]
                        for k in range(KD):
                            nc.tensor.matmul(
                                pss[:],
                                qt[:, k, bs:bs + 128].bitcast(F32R),
                                tin[:, k * STRIP:(k + 1) * STRIP].bitcast(F32R),
                                start=(k == 0), stop=(k == KD - 1),
                            )
                    else:
                        qh, ql = qts[l]
                        terms = []
                        for k in range(KD):
                            sl = slice(k * STRIP, (k + 1) * STRIP)
                            terms += [(qh[:, k, bs:bs + 128], tin_h[:, sl]),
                                      (qh[:, k, bs:bs + 128], tin_l[:, sl]),
                                      (ql[:, k, bs:bs + 128], tin_h[:, sl])]
                        for ti, (wa, xb) in enumerate(terms):
                            nc.tensor.matmul(
                                pss[:], wa, xb,
                                start=(ti == 0), stop=(ti == len(terms) - 1),
                            )
                    # PSUM eviction doubles as label packing: low 4 mantissa
                    # bits of the score are replaced by the train label, so
                    # the top-k funnel carries labels implicitly.
                    n0 = s * STRIP
                    pk = pk_pool.tile([128, STRIP], F32)
                    nc.vector.scalar_tensor_tensor(
                        out=pk[:].bitcast(U32),
                        in0=pss[:].bitcast(U32),
                        scalar=mask_hi[:],
                        in1=labb[:, n0:n0 + STRIP],
                        op0=mybir.AluOpType.bitwise_and,
                        op1=mybir.AluOpType.bitwise_or,
                    )
                    # stage-1: top-8 of each 250-chunk (global-top-75
                    # members per 250-chunk ~ Poisson(0.19); P(>8) ~ 1e-12)
                    cd = cands[l][rg]
                    c0 = s * CHUNK_TOPK
                    half = STRIP // 2
                    nc.vector.max(out=cd[:, c0:c0 + 8], in_=pk[:, :half])
                    nc.vector.max(out=cd[:, c0 + 8:c0 + 16], in_=pk[:, half:])

            # local top-32 peel for this layer, then per-layer all-gather
            # (layer 0's collective overlaps layer 1's strips)
            for rg in range(RG):
                cd, wn = cands[l][rg], wins[l][rg]
                for r in range(N_ROUNDS):
                    w8 = wn[:, r * 8:(r + 1) * 8]
                    nc.vector.max(out=w8, in_=cd[:])
                    if r < N_ROUNDS - 1:
                        nc.vector.match_replace(
                            out=cd[:], in_to_replace=w8,
                            in_values=cd[:], imm_value=NEG_INF,
                        )
                nc.sync.dma_start(out=local_cand[l][rg], in_=wn[:])

            nc.gpsimd.collective_compute(
                "AllGather",
                mybir.AluOpType.bypass,
                replica_groups=[list(range(N_CORES))],
                ins=[local_cand[l][:]],
                outs=[gshared[l][:]],
            )

        # ---------------- replicated global merge
        for rg in range(RG):
            labw = persist.tile([128, L * K], U32, name=f"labw{rg}", tag=f"labw{rg}")
            for l in range(L):
                gcand = persist.tile([128, N_CORES * N_KEEP], F32,
                                     name=f"gcand{rg}_{l}", tag=f"gcand{rg}_{l}")
                nc.sync.dma_start(
                    out=gcand[:],
                    in_=gshared[l][:, rg, :, :].rearrange("c p k -> p c k"),
                )
                gwin = persist.tile([128, K_MERGE_ROUNDS * 8], F32,
                                    name=f"gwin{rg}_{l}", tag=f"gwin{rg}_{l}")
                for r in range(K_MERGE_ROUNDS):
                    w8 = gwin[:, r * 8:(r + 1) * 8]
                    nc.vector.max(out=w8, in_=gcand[:])
                    if r < K_MERGE_ROUNDS - 1:
                        nc.vector.match_replace(
                            out=gcand[:], in_to_replace=w8,
                            in_values=gcand[:], imm_value=NEG_INF,
                        )
                # labels of the global top-75
                nc.vector.tensor_scalar(
                    out=labw[:, l * K:(l + 1) * K],
                    in0=gwin[:, 0:K].bitcast(U32),
                    scalar1=mask_lo[:], scalar2=None,
                    op0=mybir.AluOpType.bitwise_and,
                )

            # labels as f32 values for is_equal comparisons
            labwf = persist.tile([128, L * K], F32, name=f"labwf{rg}", tag=f"labwf{rg}")
            nc.vector.tensor_copy(labwf[:], labw[:])
            # per class c: count_c = #{selected labels == c};
            # m_c = #{cali >= 150 - count_c} = sum_{w<=count_c} htab[w]
            scr = persist.tile([128, L * K], F32, name=f"scr{rg}", tag=f"scr{rg}")
            vt = persist.tile([128, C], F32, name=f"vt{rg}", tag=f"vt{rg}")
            scrh = persist.tile([128, NH], F32, name=f"scrh{rg}", tag=f"scrh{rg}")
            mge = persist.tile([128, C], F32, name=f"mge{rg}", tag=f"mge{rg}")
            mp = persist.tile([128, C], F32, name=f"mp{rg}", tag=f"mp{rg}")
            for c in range(C):
                cnt = vt[:, c:c + 1]
                nc.vector.tensor_scalar(
                    out=scr[:], in0=labwf[:], scalar1=float(c), scalar2=0.0,
                    op0=mybir.AluOpType.is_equal, op1=mybir.AluOpType.add,
                    accum_out=cnt,
                )
                nc.vector.scalar_tensor_tensor(
                    out=scrh[:], in0=irow[:], scalar=cnt, in1=hrow[:],
                    op0=mybir.AluOpType.is_le, op1=mybir.AluOpType.mult,
                    accum_out=mge[:, c:c + 1],
                )
                # tie-break packing: mp = m*16 + (15 - c); argmax prefers
                # larger m then smaller class index, matching jnp.argmax
                nc.vector.tensor_scalar(
                    out=mp[:, c:c + 1], in0=mge[:, c:c + 1],
                    scalar1=16.0, scalar2=float(15 - c),
                    op0=mybir.AluOpType.mult, op1=mybir.AluOpType.add,
                )
            rmax = persist.tile([128, 1], F32, name=f"rmax{rg}", tag=f"rmax{rg}")
            nc.vector.tensor_reduce(
                out=rmax[:], in_=mp[:], axis=mybir.AxisListType.X,
                op=mybir.AluOpType.max,
            )
            crd = persist.tile([128, C], F32, name=f"crd{rg}", tag=f"crd{rg}")
            nc.vector.scalar_tensor_tensor(
                out=crd[:], in0=mp[:], scalar=rmax[:], in1=mge[:],
                op0=mybir.AluOpType.is_equal, op1=mybir.AluOpType.mult,
            )
            nc.vector.tensor_scalar(
                out=crd[:], in0=crd[:], scalar1=1.0 / NB_CALI, scalar2=None,
                op0=mybir.AluOpType.mult,
            )
            nc.sync.dma_start(out=creds[rg * 128:(rg + 1) * 128, :], in_=crd[:])

    return nc


_CACHE = {}


def _split_bf16(x):
    import ml_dtypes
    hi = x.astype(ml_dtypes.bfloat16)
    lo = (x - hi.astype(np.float32)).astype(ml_dtypes.bfloat16)
    return hi, lo


def _strip_layout(tT):
    # [L, D, N_SHARD] -> [L, N_STRIPS, 128, KD*STRIP], d=(k,p), n=(s,c)
    x = tT.reshape(tT.shape[0], KD, 128, N_STRIPS, STRIP)
    return np.ascontiguousarray(
        x.transpose(0, 3, 2, 1, 4).reshape(tT.shape[0], N_STRIPS, 128, KD * STRIP)
    )


def _q_layout(qT):
    # [L, D, B] -> [L, 128, KD*B]
    x = qT.reshape(L, KD, 128, B)
    return np.ascontiguousarray(x.transpose(0, 2, 1, 3).reshape(L, 128, KD * B))


def _prep_inputs(train_feats, query_feats, train_labels, cali_nonconformity):
    train_feats = np.asarray(train_feats, dtype=np.float32)
    query_feats = np.asarray(query_feats, dtype=np.float32)
    labels = np.asarray(train_labels).astype(np.uint32)
    cali = np.asarray(cali_nonconformity).astype(np.int64)

    # m(count) = #{cali >= 150 - count} = prefix-sum over a 151-bin histogram
    htab = np.zeros(NH, dtype=np.float32)
    for w in range(NH):
        htab[w] = np.count_nonzero(cali == (L * K - w))
    iotah = np.arange(NH, dtype=np.float32)

    that = train_feats / np.linalg.norm(train_feats, axis=-1, keepdims=True)
    qT = np.ascontiguousarray(query_feats.transpose(0, 2, 1))  # [L, D, B]

    common = {"htab": htab, "iotah": iotah}
    if SCHEME == "f32r":
        qWf = _q_layout(qT)
    else:
        qhi, qlo = _split_bf16(qT)
        qHIf, qLOf = _q_layout(qhi), _q_layout(qlo)

    in_maps = []
    for i in range(N_CORES):
        sl = slice(i * N_SHARD, (i + 1) * N_SHARD)
        tT = np.ascontiguousarray(that[:, sl, :].transpose(0, 2, 1))  # [L,D,Ns]
        m = dict(common)
        if SCHEME == "f32r":
            m["tS"] = _strip_layout(tT)
            m["qW"] = qWf
        else:
            thi, tlo = _split_bf16(tT)
            m["tHI"] = _strip_layout(thi)
            m["tLO"] = _strip_layout(tlo)
            m["qHI"] = qHIf
            m["qLO"] = qLOf
        m["labels"] = np.ascontiguousarray(labels[sl])
        in_maps.append(m)
    return in_maps


def kernel(train_feats, query_feats, train_labels, cali_nonconformity,
           trace=False, **trace_kwargs):
    key = ("nc", SCHEME)
    if key not in _CACHE:
        _CACHE[key] = build_kernel(SCHEME)
    nc = _CACHE[key]
    in_maps = _prep_inputs(
        train_feats, query_feats, train_labels, cali_nonconformity
    )
    res = run_bass_kernel_spmd(
        nc, in_maps, list(range(N_CORES)), trace=trace, **trace_kwargs
    )
    _CACHE["last_result"] = res
    return np.asarray(res.results[0]["creds"], dtype=np.float32)


# revision 12
# speedup vs baseline: 2.0858x; 1.2057x over previous
"""Distributed kNN (DkNN conformal credibility) on 8 TRN2 NeuronCores.

Math: the reference's per-layer normalize+center cancels for ranking ---
top-75 by EuclideanSquared of normalized-centered vectors == top-75 by
descending q . (t_n/||t_n||).  The host pre-normalizes the train shard, so
the kernel is a pure score matmul + top-k funnel: each core scores its
12500-point shard against all 256 queries (fp32r matmul, exact fp32),
packs the train label into the low 4 mantissa bits of the score, selects
a local top-32 per query via max8/match_replace peeling, all-gathers the
8x32 candidates per layer (layer 0's gather overlaps layer 1's compute),
and every core redundantly reduces to the global top-75, class counts,
conformal p-values (via a host-precomputed 151-entry histogram of the
calibration array), and argmax credibility.
"""

import os
import sys
import types

for _p in ("/opt/trn_rl_repo", "/root/.axon_site/_ro/trn_rl_repo"):
    if os.path.isdir(_p) and _p not in sys.path:
        sys.path.insert(0, _p)

import numpy as np

import concourse.bass as bass
import concourse.mybir as mybir
from concourse.tile import TileContext
from concourse.vector_clock import ScopedClock
from concourse.bass_utils import run_bass_kernel_spmd

# ---------------------------------------------------------------- constants
N_CORES = 8
L = 2
N_TRAIN = 100000
N_SHARD = N_TRAIN // N_CORES          # 12500
D = 512
B = 256
K = 75
C = 10
NB_CALI = 750
KD = D // 128                         # 4 contraction k-tiles
STRIP = 500                           # n-columns per strip (25 exact strips)
N_STRIPS = N_SHARD // STRIP           # 25
CHUNK_TOPK = 16                       # stage-1 candidates per strip (2x max8)
N_CAND = N_STRIPS * CHUNK_TOPK        # 400 stage-1 candidates per (layer,rg)
N_ROUNDS = 3                          # local peel rounds of 8 -> top-24
N_KEEP = N_ROUNDS * 8                 # 24 shipped per (layer,rg,row)
K_MERGE_ROUNDS = 10                   # global peel -> top-80 >= 75
NEG_INF = -3.0e38
RG = B // 128                         # 2 query row-groups
NH = 151                              # histogram table size (L*K+1)

F32 = mybir.dt.float32
F32R = mybir.dt.float32r
BF16 = mybir.dt.bfloat16
U32 = mybir.dt.uint32

SCHEME = os.environ.get("KNN_SCHEME", "f32r")   # "f32r" | "bf16x3"


# ------------------------------------------------- tile tail-drain workaround
def _patched_drain_and_barrier(self, tick_clock, wait_clock):
    # walrus rejects >few sync waits on one instruction; the stock tail
    # drain aggregates every live semaphore.  Spread them over drains.
    nc = self.nc
    drain_inst = nc.sync.drain()
    wait_clock.add_sem_waits(
        drain_inst.ins, ScopedClock({None: tick_clock.global_clock})
    )
    si = drain_inst.ins.sync_info
    waits = list(si.on_wait) if si is not None else []
    if len(waits) > 1:
        si.on_wait = waits[:1]
        SyncInfo = type(si)
        for w in waits[1:]:
            wi = nc.sync.drain()
            wi.ins.sync_info = SyncInfo(on_wait=[w], on_update=[])

    nc.all_engine_barrier()
    assert self.sems is not None
    popped = nc._tile_sem_poison_stack.pop()
    assert popped is self._sem_poison
    nc.clear_and_free_semaphores(list(self.sems.allocated().values()))
    nc.all_engine_barrier()


TileContext._drain_and_barrier = _patched_drain_and_barrier

# Cap embedded sync-waits per instruction; hoist the excess onto injected
# same-engine NOPs placed immediately before (program order on one engine
# makes this equivalent).
_MAX_WAITS = 1
_orig_lower_ordered_insts = TileContext._lower_ordered_insts


def _redistribute_waits(self, ordered):
    nc = self.nc
    SyncInfo = None
    for bb_name, insts in ordered.items():
        new_list = []
        for inst in insts:
            si = getattr(inst, 'sync_info', None)
            waits = list(si.on_wait) if si is not None else []
            cap = 1 if not isinstance(inst, mybir.InstNoOp) else _MAX_WAITS
            if len(waits) > cap:
                if SyncInfo is None:
                    SyncInfo = type(si)
                keep = waits[:cap]
                extra = waits[cap:]
                si.on_wait = keep
                for j in range(0, len(extra), _MAX_WAITS):
                    nop = mybir.InstNoOp(
                        name=f"waitnop-{nc.next_id()}", ins=[], outs=[],
                        engine=inst.engine,
                    )
                    nop.sync_info = SyncInfo(
                        on_wait=extra[j:j + _MAX_WAITS], on_update=[]
                    )
                    nc.register_instruction(nop, overwrite=True)
                    new_list.append(nop)
            new_list.append(inst)
        insts[:] = new_list
    return _orig_lower_ordered_insts(self, ordered)


TileContext._lower_ordered_insts = _redistribute_waits


def _register_ntff_hook():
    """Optional: make run_bass_kernel_spmd(trace=True) work under axon."""
    if 'antenv.axon_hooks' not in sys.modules:
        m = types.ModuleType('antenv.axon_hooks')
        hook = [None]
        m.set_axon_ntff_profile_hook = lambda h: hook.__setitem__(0, h)
        m.get_axon_ntff_profile_hook = lambda: hook[0]
        sys.modules['antenv.axon_hooks'] = m
        try:
            import antenv
            antenv.axon_hooks = m
        except ImportError:
            pass
    try:
        from antenv.axon_hooks import set_axon_ntff_profile_hook
        from trn_agent_boot.trn_boot import _ntff_profile_via_ctypes
        h = _ntff_profile_via_ctypes('/opt/axon/libaxon_pjrt.so')
        if h is not None:
            set_axon_ntff_profile_hook(h)
    except Exception:
        pass


_register_ntff_hook()


# ------------------------------------------------------------- kernel build
def build_kernel(scheme):
    nc = bass.Bass("TRN2", num_devices=N_CORES)

    if scheme == "f32r":
        tS = nc.declare_dram_parameter(
            "tS", [L, N_STRIPS, 128, KD * STRIP], F32, isOutput=False)
        qW = nc.declare_dram_parameter("qW", [L, 128, KD * B], F32, isOutput=False)
    else:
        tHI = nc.declare_dram_parameter(
            "tHI", [L, N_STRIPS, 128, KD * STRIP], BF16, isOutput=False)
        tLO = nc.declare_dram_parameter(
            "tLO", [L, N_STRIPS, 128, KD * STRIP], BF16, isOutput=False)
        qHI = nc.declare_dram_parameter("qHI", [L, 128, KD * B], BF16, isOutput=False)
        qLO = nc.declare_dram_parameter("qLO", [L, 128, KD * B], BF16, isOutput=False)
    labels = nc.declare_dram_parameter("labels", [N_SHARD], U32, isOutput=False)
    htab = nc.declare_dram_parameter("htab", [NH], F32, isOutput=False)
    iotah = nc.declare_dram_parameter("iotah", [NH], F32, isOutput=False)
    creds = nc.declare_dram_parameter("creds", [B, C], F32, isOutput=True)

    local_cand = [
        nc.dram_tensor(f"local_cand{l}", [RG, 128, N_KEEP], F32) for l in range(L)
    ]
    gshared = [
        nc.dram_tensor(
            f"gshared{l}", [N_CORES, RG, 128, N_KEEP], F32, addr_space="Shared"
        )
        for l in range(L)
    ]

    with TileContext(nc) as tc, \
         tc.tile_pool(name="persist", bufs=1) as persist, \
         tc.tile_pool(name="tin", bufs=12) as tin_pool, \
         tc.tile_pool(name="pk", bufs=3) as pk_pool, \
         tc.tile_pool(name="pss", bufs=4, space="PSUM") as pss_pool:

        # ---------------- setup (all overlappable with early strips)
        # labels are broadcast per strip inside the layer-0 loop so the bulk
        # transfer never gates the first PSUM eviction
        labb = persist.tile([128, N_SHARD], U32)
        hrow = persist.tile([128, NH], F32)
        nc.scalar.dma_start(out=hrow[:], in_=htab[:].partition_broadcast(128))
        irow = persist.tile([128, NH], F32)
        nc.scalar.dma_start(out=irow[:], in_=iotah[:].partition_broadcast(128))
        mask_hi = persist.tile([128, 1], U32)
        nc.vector.memset(mask_hi[:], 0xFFFFFFF0)
        mask_lo = persist.tile([128, 1], U32)
        nc.vector.memset(mask_lo[:], 0xF)

        qts = []
        for l in range(L):
            if scheme == "f32r":
                qt = persist.tile([128, KD, B], F32, name=f"qt{l}", tag=f"qt{l}")
                nc.sync.dma_start(out=qt[:], in_=qW[l])
                qts.append((qt,))
            else:
                qh = persist.tile([128, KD, B], BF16, name=f"qth{l}", tag=f"qth{l}")
                nc.sync.dma_start(out=qh[:], in_=qHI[l])
                ql = persist.tile([128, KD, B], BF16, name=f"qtl{l}", tag=f"qtl{l}")
                nc.sync.dma_start(out=ql[:], in_=qLO[l])
                qts.append((qh, ql))

        cands = [
            [persist.tile([128, N_CAND], F32, name=f"cand{l}_{rg}",
                          tag=f"cand{l}_{rg}") for rg in range(RG)]
            for l in range(L)
        ]
        wins = [
            [persist.tile([128, N_KEEP], F32, name=f"win{l}_{rg}",
                          tag=f"win{l}_{rg}") for rg in range(RG)]
            for l in range(L)
        ]

        labws = []

        def emit_strip(l, s):
            n0 = s * STRIP
            if l == 0:
                # JIT per-strip label broadcast, off the critical DMA queues
                nc.gpsimd.dma_start(
                    out=labb[:, n0:n0 + STRIP],
                    in_=labels[n0:n0 + STRIP].partition_broadcast(128),
                )
            if scheme == "f32r":
                tin = tin_pool.tile([128, KD * STRIP], F32, name="tin",
                                    tag="tin")
                eng = nc.sync if s % 2 == 0 else nc.scalar
                eng.dma_start(out=tin[:], in_=tS[l, s])
            else:
                tin_h = tin_pool.tile([128, KD * STRIP], BF16, name="tin_h",
                                      tag="tin_h")
                nc.sync.dma_start(out=tin_h[:], in_=tHI[l, s])
                tin_l = tin_pool.tile([128, KD * STRIP], BF16, name="tin_l",
                                      tag="tin_l")
                nc.scalar.dma_start(out=tin_l[:], in_=tLO[l, s])

            # both row-groups interleaved per term: each weight load (lhsT)
            # feeds two 500-column streams, halving LDWEIGHTS overhead
            pss = [pss_pool.tile([128, STRIP], F32, name=f"pss{rg}",
                                 tag=f"pss{rg}") for rg in range(RG)]
            if scheme == "f32r":
                (qt,) = qts[l]
                terms = [(qt[:, k, :], tin[:, k * STRIP:(k + 1) * STRIP])
                         for k in range(KD)]
            else:
                qh, ql = qts[l]
                terms = []
                for k in range(KD):
                    sl = slice(k * STRIP, (k + 1) * STRIP)
                    terms += [(qh[:, k, :], tin_h[:, sl]),
                              (qh[:, k, :], tin_l[:, sl]),
                              (ql[:, k, :], tin_h[:, sl])]
            for ti, (wq, xb) in enumerate(terms):
                for rg in range(RG):
                    bs = rg * 128
                    wa = wq[:, bs:bs + 128]
                    if scheme == "f32r":
                        wa, xb2 = wa.bitcast(F32R), xb.bitcast(F32R)
                    else:
                        xb2 = xb
                    nc.tensor.matmul(
                        pss[rg][:], wa, xb2,
                        start=(ti == 0), stop=(ti == len(terms) - 1),
                    )
            for rg in range(RG):
                # PSUM eviction doubles as label packing: low 4 mantissa
                # bits of the score are replaced by the train label, so
                # the top-k funnel carries labels implicitly.
                pk = pk_pool.tile([128, STRIP], F32, name="pk", tag="pk")
                nc.vector.scalar_tensor_tensor(
                    out=pk[:].bitcast(U32),
                    in0=pss[rg][:].bitcast(U32),
                    scalar=mask_hi[:],
                    in1=labb[:, n0:n0 + STRIP],
                    op0=mybir.AluOpType.bitwise_and,
                    op1=mybir.AluOpType.bitwise_or,
                )
                # stage-1: top-8 of each 250-chunk (global-top-75
                # members per 250-chunk ~ Poisson(0.19); P(>8) ~ 1e-12)
                cd = cands[l][rg]
                c0 = s * CHUNK_TOPK
                half = STRIP // 2
                nc.vector.max(out=cd[:, c0:c0 + 8], in_=pk[:, :half])
                nc.vector.max(out=cd[:, c0 + 8:c0 + 16], in_=pk[:, half:])

        def emit_tail(l):
            # local top-24 peel for this layer, then per-layer all-gather
            # (layer 0's collective overlaps layer 1's strips)
            for rg in range(RG):
                cd, wn = cands[l][rg], wins[l][rg]
                for r in range(N_ROUNDS):
                    w8 = wn[:, r * 8:(r + 1) * 8]
                    nc.vector.max(out=w8, in_=cd[:])
                    if r < N_ROUNDS - 1:
                        nc.vector.match_replace(
                            out=cd[:], in_to_replace=w8,
                            in_values=cd[:], imm_value=NEG_INF,
                        )
                nc.sync.dma_start(out=local_cand[l][rg], in_=wn[:])

            nc.gpsimd.collective_compute(
                "AllGather",
                mybir.AluOpType.bypass,
                replica_groups=[list(range(N_CORES))],
                ins=[local_cand[l][:]],
                outs=[gshared[l][:]],
            )

        def emit_merge(l):
            # replicated global merge for one layer's gathered candidates.
            # gcand loads go through gpsimd so a late collective can only
            # stall the gpsimd queue, never the strip-load queues.
            for rg in range(RG):
                if l == 0:
                    labws.append(persist.tile(
                        [128, L * K], U32, name=f"labw{rg}", tag=f"labw{rg}"))
                labw = labws[rg]
                gcand = persist.tile([128, N_CORES * N_KEEP], F32,
                                     name=f"gcand{rg}_{l}", tag=f"gcand{rg}_{l}")
                nc.gpsimd.dma_start(
                    out=gcand[:],
                    in_=gshared[l][:, rg, :, :].rearrange("c p k -> p c k"),
                )
                gwin = persist.tile([128, K_MERGE_ROUNDS * 8], F32,
                                    name=f"gwin{rg}_{l}", tag=f"gwin{rg}_{l}")
                for r in range(K_MERGE_ROUNDS):
                    w8 = gwin[:, r * 8:(r + 1) * 8]
                    nc.vector.max(out=w8, in_=gcand[:])
                    if r < K_MERGE_ROUNDS - 1:
                        nc.vector.match_replace(
                            out=gcand[:], in_to_replace=w8,
                            in_values=gcand[:], imm_value=NEG_INF,
                        )
                # labels of the global top-75
                nc.vector.tensor_scalar(
                    out=labw[:, l * K:(l + 1) * K],
                    in0=gwin[:, 0:K].bitcast(U32),
                    scalar1=mask_lo[:], scalar2=None,
                    op0=mybir.AluOpType.bitwise_and,
                )

        def emit_counts(rg, eng):
            # per class c: count_c = #{selected labels == c};
            # m_c = #{cali >= 150 - count_c} = sum_{w<=count_c} htab[w]
            labwf = persist.tile([128, L * K], F32, name=f"labwf{rg}", tag=f"labwf{rg}")
            eng.tensor_copy(labwf[:], labws[rg][:])
            scr = persist.tile([128, L * K], F32, name=f"scr{rg}", tag=f"scr{rg}")
            vt = persist.tile([128, C], F32, name=f"vt{rg}", tag=f"vt{rg}")
            scrh = persist.tile([128, NH], F32, name=f"scrh{rg}", tag=f"scrh{rg}")
            mge = persist.tile([128, C], F32, name=f"mge{rg}", tag=f"mge{rg}")
            mp = persist.tile([128, C], F32, name=f"mp{rg}", tag=f"mp{rg}")
            for c in range(C):
                cnt = vt[:, c:c + 1]
                eng.tensor_scalar(
                    out=scr[:], in0=labwf[:], scalar1=float(c), scalar2=0.0,
                    op0=mybir.AluOpType.is_equal, op1=mybir.AluOpType.add,
                    accum_out=cnt,
                )
                eng.scalar_tensor_tensor(
                    out=scrh[:], in0=irow[:], scalar=cnt, in1=hrow[:],
                    op0=mybir.AluOpType.is_le, op1=mybir.AluOpType.mult,
                    accum_out=mge[:, c:c + 1],
                )
                # tie-break packing: mp = m*16 + (15 - c); argmax prefers
                # larger m then smaller class index, matching jnp.argmax
                eng.tensor_scalar(
                    out=mp[:, c:c + 1], in0=mge[:, c:c + 1],
                    scalar1=16.0, scalar2=float(15 - c),
                    op0=mybir.AluOpType.mult, op1=mybir.AluOpType.add,
                )
            # row-max over the 10 classes via a small max tree (gpsimd has
            # no free-axis tensor_reduce)
            t5 = persist.tile([128, C // 2], F32, name=f"t5{rg}", tag=f"t5{rg}")
            eng.tensor_tensor(out=t5[:], in0=mp[:, 0:5], in1=mp[:, 5:10],
                              op=mybir.AluOpType.max)
            rmax = persist.tile([128, 1], F32, name=f"rmax{rg}", tag=f"rmax{rg}")
            eng.tensor_tensor(out=rmax[:], in0=t5[:, 0:1], in1=t5[:, 1:2],
                              op=mybir.AluOpType.max)
            for j in range(2, 5):
                eng.tensor_tensor(out=rmax[:], in0=rmax[:], in1=t5[:, j:j + 1],
                                  op=mybir.AluOpType.max)
            crd = persist.tile([128, C], F32, name=f"crd{rg}", tag=f"crd{rg}")
            eng.scalar_tensor_tensor(
                out=crd[:], in0=mp[:], scalar=rmax[:], in1=mge[:],
                op0=mybir.AluOpType.is_equal, op1=mybir.AluOpType.mult,
            )
            eng.tensor_scalar(
                out=crd[:], in0=crd[:], scalar1=1.0 / NB_CALI, scalar2=None,
                op0=mybir.AluOpType.mult,
            )
            nc.sync.dma_start(out=creds[rg * 128:(rg + 1) * 128, :], in_=crd[:])

        # ---------------- main schedule
        for s in range(N_STRIPS):
            emit_strip(0, s)
        emit_tail(0)
        for s in range(N_STRIPS):
            emit_strip(1, s)
            if s == 17:
                # layer-0 gather landed long ago; merge it under the
                # remaining layer-1 strips
                emit_merge(0)
        emit_tail(1)
        emit_merge(1)
        emit_counts(0, nc.vector)
        emit_counts(1, nc.vector)

    return nc


_CACHE = {}


def _split_bf16(x):
    import ml_dtypes
    hi = x.astype(ml_dtypes.bfloat16)
    lo = (x - hi.astype(np.float32)).astype(ml_dtypes.bfloat16)
    return hi, lo


def _strip_layout(tT):
    # [L, D, N_SHARD] -> [L, N_STRIPS, 128, KD*STRIP], d=(k,p), n=(s,c)
    x = tT.reshape(tT.shape[0], KD, 128, N_STRIPS, STRIP)
    return np.ascontiguousarray(
        x.transpose(0, 3, 2, 1, 4).reshape(tT.shape[0], N_STRIPS, 128, KD * STRIP)
    )


def _q_layout(qT):
    # [L, D, B] -> [L, 128, KD*B]
    x = qT.reshape(L, KD, 128, B)
    return np.ascontiguousarray(x.transpose(0, 2, 1, 3).reshape(L, 128, KD * B))


def _prep_inputs(train_feats, query_feats, train_labels, cali_nonconformity):
    train_feats = np.asarray(train_feats, dtype=np.float32)
    query_feats = np.asarray(query_feats, dtype=np.float32)
    labels = np.asarray(train_labels).astype(np.uint32)
    cali = np.asarray(cali_nonconformity).astype(np.int64)

    # m(count) = #{cali >= 150 - count} = prefix-sum over a 151-bin histogram
    htab = np.zeros(NH, dtype=np.float32)
    for w in range(NH):
        htab[w] = np.count_nonzero(cali == (L * K - w))
    iotah = np.arange(NH, dtype=np.float32)

    that = train_feats / np.linalg.norm(train_feats, axis=-1, keepdims=True)
    qT = np.ascontiguousarray(query_feats.transpose(0, 2, 1))  # [L, D, B]

    common = {"htab": htab, "iotah": iotah}
    if SCHEME == "f32r":
        qWf = _q_layout(qT)
    else:
        qhi, qlo = _split_bf16(qT)
        qHIf, qLOf = _q_layout(qhi), _q_layout(qlo)

    in_maps = []
    for i in range(N_CORES):
        sl = slice(i * N_SHARD, (i + 1) * N_SHARD)
        tT = np.ascontiguousarray(that[:, sl, :].transpose(0, 2, 1))  # [L,D,Ns]
        m = dict(common)
        if SCHEME == "f32r":
            m["tS"] = _strip_layout(tT)
            m["qW"] = qWf
        else:
            thi, tlo = _split_bf16(tT)
            m["tHI"] = _strip_layout(thi)
            m["tLO"] = _strip_layout(tlo)
            m["qHI"] = qHIf
            m["qLO"] = qLOf
        m["labels"] = np.ascontiguousarray(labels[sl])
        in_maps.append(m)
    return in_maps


def kernel(train_feats, query_feats, train_labels, cali_nonconformity,
           trace=False, **trace_kwargs):
    key = ("nc", SCHEME)
    if key not in _CACHE:
        _CACHE[key] = build_kernel(SCHEME)
    nc = _CACHE[key]
    in_maps = _prep_inputs(
        train_feats, query_feats, train_labels, cali_nonconformity
    )
    res = run_bass_kernel_spmd(
        nc, in_maps, list(range(N_CORES)), trace=trace, **trace_kwargs
    )
    _CACHE["last_result"] = res
    return np.asarray(res.results[0]["creds"], dtype=np.float32)
